# revision 1
# baseline (speedup 1.0000x reference)
"""GNN edge-softmax attention kernel for 8 Trainium2 NeuronCores.

Strategy (4 src-rows x 2 dst-halves core grid):
  - Host routes each edge to core (row(src), half(dst)). Nodes are packed
    into 128-node tiles balanced by edge count; each tile's edges are padded
    to whole 128-edge blocks so every core runs an identical program.
  - Per core: project q/k/v slices with PE into f16 tables in HBM, then for
    each node tile: gather qh[src], khv[dst] rows with dma_gather, compute
    per-edge logits (DVE mult + grouped reduce, PE for the edge-feature
    term), exp on ACT, and scatter-accumulate numerator/denominator into
    PSUM with selection-matrix matmuls.
  - The two dst-halves of each src row AllReduce their partial num/den
    (pairwise), then each core normalizes and applies the output projection
    for half of the tiles.
"""

import math
import sys

import numpy as np

sys.path.insert(0, "/opt/trn_rl_repo")

import concourse.bacc as bacc
import concourse.bass as bass
import concourse.mybir as mybir
import concourse.tile as tile
from concourse import bass_utils

F16 = mybir.dt.float16
F8 = mybir.dt.float8e4
F32 = mybir.dt.float32
I16 = mybir.dt.int16

H = 8            # heads
D = 16           # head dim
TD = H * D       # 128
QD = 256         # q/k/v feature dim
PD = 64          # edge pair feature dim
R = 4            # src rows of the core grid
C = 2            # dst cols of the core grid
P = 128

AF = mybir.ActivationFunctionType
ALU = mybir.AluOpType


def _wrap16(idx: np.ndarray) -> np.ndarray:
    """dma_gather index layout: [128, n/16] with idx i at (i%16 + 16k, i//16)."""
    n = idx.shape[0]
    assert n % 16 == 0
    w = idx.reshape(n // 16, 16).T.astype(np.int16)  # [16, n/16]
    return np.tile(w, (8, 1))  # replicate across the 8 partition groups


def prepare(q, k, v, edges, edge_index, Wq, Wk, Wv, Wb, bb, Wo, bo):
    N = q.shape[0]
    E = edges.shape[0]
    ntiles_row = math.ceil(N / (R * P))          # tiles per src row
    NROW = ntiles_row * P                        # nodes per row (padded)
    NPAD = NROW * R
    DHALF = NPAD // 2                            # dst-half size
    assert DHALF < 32768, "dst half must fit int16"

    src = np.asarray(edge_index[:, 0], dtype=np.int64)
    dst = np.asarray(edge_index[:, 1], dtype=np.int64)
    deg = np.bincount(src, minlength=N)

    # --- greedy node->tile packing balanced by edge count ---
    T = R * ntiles_row
    order = np.argsort(-deg, kind="stable")
    tile_cnt = np.zeros(T, dtype=np.int64)       # nodes in tile
    tile_edges = np.zeros(T, dtype=np.int64)
    node_tile = np.zeros(N, dtype=np.int32)
    node_slot = np.zeros(N, dtype=np.int32)
    import heapq
    heap = [(0, t) for t in range(T)]
    heapq.heapify(heap)
    for n in order:
        while True:
            e_cnt, t = heapq.heappop(heap)
            if tile_cnt[t] < P:
                break
        node_tile[n] = t
        node_slot[n] = tile_cnt[t]
        tile_cnt[t] += 1
        tile_edges[t] += deg[n]
        if tile_cnt[t] < P:
            heapq.heappush(heap, (tile_edges[t], t))

    row_of_edge = node_tile[src] // ntiles_row
    j_of_edge = (dst // DHALF).astype(np.int64)
    tloc_of_edge = (node_tile[src] % ntiles_row).astype(np.int64)

    # per (core, tile_local) edge counts -> shared block counts per tile slot
    core_of_edge = row_of_edge * C + j_of_edge
    cnt = np.zeros((R * C, ntiles_row), dtype=np.int64)
    np.add.at(cnt, (core_of_edge, tloc_of_edge), 1)
    bpt = np.maximum(1, np.ceil(cnt.max(axis=0) / P).astype(np.int64))  # [ntiles_row]
    blk_off = np.concatenate([[0], np.cumsum(bpt)])   # block offset per tile
    NBLK = int(blk_off[-1])
    ECAP = NBLK * P

    # --- per-core edge arrays ---
    cores = []
    eT_all = np.asarray(edges, dtype=np.float32).T    # [PD, E]
    for core in range(R * C):
        i, j = core // C, core % C
        mask = core_of_edge == core
        es, ed, et = src[mask], dst[mask], tloc_of_edge[mask]
        # order edges by tile slot
        ordr = np.argsort(et, kind="stable")
        es, ed, et = es[ordr], ed[ordr], et[ordr]
        # positions: per tile, fill from blk_off[t]*P
        pos = np.zeros(len(es), dtype=np.int64)
        start = 0
        for t in range(ntiles_row):
            c = int((et == t).sum())
            pos[start:start + c] = blk_off[t] * P + np.arange(c)
            start += c
        eidx = np.nonzero(mask)[0][ordr]

        import ml_dtypes
        F8NP = ml_dtypes.float8_e4m3
        dst_local = np.zeros(ECAP, dtype=np.int16)
        src_rel = np.full(ECAP, 255, dtype=np.int64)
        edgesT = np.zeros((PD + 1, ECAP), dtype=np.float16)
        edgesT[PD, :] = 1.0
        dst_local[pos] = (ed - j * DHALF).astype(np.int16)
        src_rel[pos] = node_slot[es]
        edgesT[:PD, pos] = eT_all[:, eidx].astype(np.float16)
        # one-hot selection matrices (fp8, exact 0/1)
        S_en = np.zeros((ECAP, P), dtype=F8NP)
        valid = src_rel < P
        S_en[np.nonzero(valid)[0], src_rel[valid]] = 1.0
        S_en3 = S_en.reshape(NBLK, P, P)                       # [b, e, n]
        S_mat = np.ascontiguousarray(S_en3.transpose(1, 0, 2)).reshape(P, ECAP)   # [e_part, (b n)]
        ST_mat = np.ascontiguousarray(S_en3.transpose(2, 0, 1)).reshape(P, ECAP)  # [n_part, (b e)]

        # constants: this core projects quarter i of half j's khv table
        Q4 = DHALF // 4
        qlo = j * DHALF + i * Q4
        qhi = min(qlo + Q4, N)
        kT = np.zeros((QD, Q4), dtype=np.float16)
        vT = np.zeros((QD, Q4), dtype=np.float16)
        if qhi > qlo:
            kT[:, :qhi - qlo] = np.asarray(k[qlo:qhi], np.float32).T.astype(np.float16)
            vT[:, :qhi - qlo] = np.asarray(v[qlo:qhi], np.float32).T.astype(np.float16)
        # q rows permuted into (tile_local, slot) order for this row i
        qT = np.zeros((QD, NROW), dtype=np.float16)
        rmask = node_tile // ntiles_row == i
        rn = np.nonzero(rmask)[0]
        qpos = (node_tile[rn] % ntiles_row) * P + node_slot[rn]
        qT[:, qpos] = np.asarray(q[rn], np.float32).T.astype(np.float16)

        cores.append(dict(
            dst_idx=_wrap16(dst_local), S_mat=S_mat, ST_mat=ST_mat,
            edgesT=edgesT, kT=kT, vT=vT, qT=qT,
        ))

    norm = D ** -0.5
    consts = dict(
        WkT=np.asarray(Wk, np.float32).T.astype(np.float16),
        WvT=np.asarray(Wv, np.float32).T.astype(np.float16),
        WqT=(np.asarray(Wq, np.float32) * norm).T.astype(np.float16),
        WbT_aug=np.concatenate(
            [np.asarray(Wb, np.float32).T,
             np.asarray(bb, np.float32)[None, :]], axis=0).astype(np.float16),
        WoT=np.asarray(Wo, np.float32).T.astype(np.float16),
        bo_row=np.asarray(bo, np.float32)[None, :].astype(np.float16),
        identity=np.eye(P, dtype=np.float16),
        ones_col=np.ones((1, P), dtype=np.float16),
    )
    meta = dict(N=N, NPAD=NPAD, NROW=NROW, DHALF=DHALF, ntiles_row=ntiles_row,
                NBLK=NBLK, ECAP=ECAP, bpt=bpt.tolist(), blk_off=blk_off.tolist(),
                node_tile=node_tile, node_slot=node_slot, deg=deg)
    return cores, consts, meta


def build_program(meta, gather_batch=3):
    """Build the SPMD bass program. Returns compiled nc."""
    ntr = meta["ntiles_row"]
    NROW, DHALF = meta["NROW"], meta["DHALF"]
    NBLK, ECAP = meta["NBLK"], meta["ECAP"]
    bpt, blk_off = meta["bpt"], meta["blk_off"]
    NKV = DHALF // P      # khv chunks
    NQ = NROW // P        # qh chunks
    half_t = ntr // 2     # tiles finalized per core

    nc = bacc.Bacc("TRN2", target_bir_lowering=False, debug=False, num_devices=R * C)
    dt = nc.dram_tensor
    # inputs
    t_dst = dt("dst_idx", [P, ECAP // 16], I16, kind="ExternalInput").ap()
    t_S = dt("S_mat", [P, ECAP], F8, kind="ExternalInput").ap()
    t_ST = dt("ST_mat", [P, ECAP], F8, kind="ExternalInput").ap()
    t_eT = dt("edgesT", [PD + 1, ECAP], F16, kind="ExternalInput").ap()
    t_kT = dt("kT", [QD, DHALF // 4], F16, kind="ExternalInput").ap()
    t_vT = dt("vT", [QD, DHALF // 4], F16, kind="ExternalInput").ap()
    t_qT = dt("qT", [QD, NROW], F16, kind="ExternalInput").ap()
    t_WkT = dt("WkT", [QD, TD], F16, kind="ExternalInput").ap()
    t_WvT = dt("WvT", [QD, TD], F16, kind="ExternalInput").ap()
    t_WqT = dt("WqT", [QD, TD], F16, kind="ExternalInput").ap()
    t_Wb = dt("WbT_aug", [PD + 1, H], F16, kind="ExternalInput").ap()
    t_WoT = dt("WoT", [TD, QD], F16, kind="ExternalInput").ap()
    t_bo = dt("bo_row", [1, QD], F16, kind="ExternalInput").ap()
    t_id = dt("identity", [P, P], F16, kind="ExternalInput").ap()
    t_ones = dt("ones_col", [1, P], F16, kind="ExternalInput").ap()
    # internal DRAM
    t_khv = dt("khv_tab", [DHALF, 2 * TD], F16).ap()
    t_khv_half = dt("khv_half", [DHALF // 4, 2 * TD], F16).ap()
    H1 = ntr                    # single tail reduce (overlap split regressed)
    t_nd_a = dt("nd_part_a", [H1 * P, 136], F32).ap()
    t_ndr_a = dt("nd_red_a", [H1 * P, 136], F32).ap()
    t_nd_b = dt("nd_part_b", [max(ntr - H1, 1) * P, 136], F32).ap()
    t_ndr_b = dt("nd_red_b", [max(ntr - H1, 1) * P, 136], F32).ap()
    # output (every core finalizes all tiles of its row; host reads j=0 cores)
    t_out = dt("o_out", [ntr * P, QD], F32, kind="ExternalOutput").ap()

    GB = gather_batch

    with tile.TileContext(nc) as tc:
        with (
            tc.tile_pool(name="const", bufs=1) as cpool,
            tc.tile_pool(name="proj", bufs=3) as ppool,
            tc.tile_pool(name="gath", bufs=3) as gpool,
            tc.tile_pool(name="work", bufs=2) as wpool,
            tc.tile_pool(name="out", bufs=2) as opool,
            tc.tile_pool(name="psA", bufs=2, space="PSUM") as psA,
            tc.tile_pool(name="psB", bufs=2, space="PSUM") as psB,
            tc.tile_pool(name="psC", bufs=1, space="PSUM") as psC,
        ):
            # ---- constants to SBUF ----
            c_WkT = cpool.tile([P, 2 * TD], F16)
            nc.sync.dma_start(out=c_WkT[:, 0:TD], in_=t_WkT[0:P, :])
            nc.sync.dma_start(out=c_WkT[:, TD:2 * TD], in_=t_WkT[P:QD, :])
            c_WvT = cpool.tile([P, 2 * TD], F16)
            nc.sync.dma_start(out=c_WvT[:, 0:TD], in_=t_WvT[0:P, :])
            nc.sync.dma_start(out=c_WvT[:, TD:2 * TD], in_=t_WvT[P:QD, :])
            c_WqT = cpool.tile([P, 2 * TD], F16)
            nc.sync.dma_start(out=c_WqT[:, 0:TD], in_=t_WqT[0:P, :])
            nc.sync.dma_start(out=c_WqT[:, TD:2 * TD], in_=t_WqT[P:QD, :])
            c_Wb = cpool.tile([PD + 1, H], F16); nc.sync.dma_start(out=c_Wb[:], in_=t_Wb)
            c_WoT = cpool.tile([TD, QD], F16); nc.sync.dma_start(out=c_WoT[:], in_=t_WoT)
            c_bo = cpool.tile([1, QD], F16); nc.sync.dma_start(out=c_bo[:], in_=t_bo)
            c_id = cpool.tile([P, P], F16); nc.sync.dma_start(out=c_id[:], in_=t_id)
            c_ones = cpool.tile([1, P], F16); nc.sync.dma_start(out=c_ones[:], in_=t_ones)
            c_dsti = cpool.tile([P, ECAP // 16], I16)
            nc.sync.dma_start(out=c_dsti[:], in_=t_dst)
            qh_sb = cpool.tile([P, NQ * TD], F16)

            # ---- phase A: projections (qh -> SBUF table, khv -> HBM) ----
            def project_qh():
                for g0 in range(0, NQ, 8):
                    g1 = min(g0 + 8, NQ)
                    w = (g1 - g0) * P
                    ina = ppool.tile([P, 1024], F16, tag="ina")
                    inb = ppool.tile([P, 1024], F16, tag="inb")
                    nc.sync.dma_start(out=ina[:, :w], in_=t_qT[0:P, g0 * P:g0 * P + w])
                    nc.sync.dma_start(out=inb[:, :w], in_=t_qT[P:QD, g0 * P:g0 * P + w])
                    for cc in range(g0, g1):
                        o = (cc - g0) * P
                        ps = psA.tile([P, TD], F32, tag="proj")
                        nc.tensor.matmul(out=ps[:], lhsT=ina[:, o:o + P], rhs=c_WqT[:, 0:TD],
                                         start=True, stop=False)
                        nc.tensor.matmul(out=ps[:], lhsT=inb[:, o:o + P], rhs=c_WqT[:, TD:2 * TD],
                                         start=False, stop=True)
                        nc.scalar.activation(out=qh_sb[:, cc * TD:(cc + 1) * TD],
                                             in_=ps[:], func=AF.Copy)

            # khv table: interleave kh|vh per node row
            def project_khv():
                # each pair member computes half the chunks into khv_half;
                # pairwise AllGather assembles the full table in t_khv.
                # SPMD trick: member j projects dst rows [j*half, (j+1)*half)
                # of ITS half -- but j differs per core while the program is
                # shared, so instead each core projects the SAME local chunk
                # range of its own kT/vT slice; the host hands core (i,j) the
                # kT/vT columns for the half it owns.
                for g0 in range(0, NKV // 4, 8):
                    g1 = min(g0 + 8, NKV // 4)
                    w = (g1 - g0) * P
                    ka = ppool.tile([P, 1024], F16, tag="ka")
                    kb = ppool.tile([P, 1024], F16, tag="kb")
                    va = ppool.tile([P, 1024], F16, tag="va")
                    vb = ppool.tile([P, 1024], F16, tag="vb")
                    nc.sync.dma_start(out=ka[:, :w], in_=t_kT[0:P, g0 * P:g0 * P + w])
                    nc.sync.dma_start(out=kb[:, :w], in_=t_kT[P:QD, g0 * P:g0 * P + w])
                    nc.sync.dma_start(out=va[:, :w], in_=t_vT[0:P, g0 * P:g0 * P + w])
                    nc.sync.dma_start(out=vb[:, :w], in_=t_vT[P:QD, g0 * P:g0 * P + w])
                    stage = ppool.tile([P, 8 * 2 * TD], F16, tag="kvstage")
                    for cc in range(g0, g1):
                        o = (cc - g0) * P
                        ps = psA.tile([P, TD], F32, tag="proj")
                        nc.tensor.matmul(out=ps[:], lhsT=ka[:, o:o + P], rhs=c_WkT[:, 0:TD],
                                         start=True, stop=False)
                        nc.tensor.matmul(out=ps[:], lhsT=kb[:, o:o + P], rhs=c_WkT[:, TD:2 * TD],
                                         start=False, stop=True)
                        nc.scalar.activation(out=stage[:, (cc - g0) * 256:(cc - g0) * 256 + TD],
                                             in_=ps[:], func=AF.Copy)
                        ps2 = psA.tile([P, TD], F32, tag="proj")
                        nc.tensor.matmul(out=ps2[:], lhsT=va[:, o:o + P], rhs=c_WvT[:, 0:TD],
                                         start=True, stop=False)
                        nc.tensor.matmul(out=ps2[:], lhsT=vb[:, o:o + P], rhs=c_WvT[:, TD:2 * TD],
                                         start=False, stop=True)
                        nc.scalar.activation(
                            out=stage[:, (cc - g0) * 256 + TD:(cc - g0) * 256 + 2 * TD],
                            in_=ps2[:], func=AF.Copy)
                    nc.sync.dma_start(
                        out=t_khv_half[g0 * P:g1 * P, :].rearrange("(c p) w -> p c w", p=P),
                        in_=stage[:, :(g1 - g0) * 256].rearrange(
                            "p (c w) -> p c w", w=256))

            import os as _os
            STAGE = _os.environ.get("KERNEL_STAGE", "full")
            project_khv()
            nc.gpsimd.collective_compute(
                "AllGather", ALU.bypass,
                replica_groups=[[0, 2, 4, 6], [1, 3, 5, 7]],
                ins=[t_khv_half], outs=[t_khv])
            project_qh()

            def reduce_half(t_in, t_out):
                if STAGE in ("proj", "gather", "attn", "scatter"):
                    return
                if _os.environ.get("KERNEL_NO_CC"):
                    nc.sync.dma_start(out=t_out, in_=t_in)
                else:
                    nc.gpsimd.collective_compute(
                        "AllReduce", ALU.add,
                        replica_groups=[[0, 1], [2, 3], [4, 5], [6, 7]],
                        ins=[t_in], outs=[t_out])

            # ---- finalize: normalize + output projection for my half ----
            # core parity selects which half of tiles via partition_id? SPMD:
            # both cores in a pair compute the same halves? No -- we give each
            # core its own tile range through the *input* side: half_sel DRAM.
            # Simpler: core j finalizes tiles [j*half_t, (j+1)*half_t) -- but j
            # differs per core with identical programs. Use partition id
            # tensor: not available here; instead finalize ALL tiles and write
            # full output; host picks the half it needs per core.
            def finalize(t):
                src_nd = t_ndr_a if t < H1 else t_ndr_b
                tt = t if t < H1 else t - H1
                ndl = opool.tile([P, 136], F32, tag="ndl")
                nc.sync.dma_start(out=ndl[:], in_=src_nd[tt * P:(tt + 1) * P, :])
                rden = opool.tile([P, H], F32, tag="rden")
                # +eps so empty node slots yield 0 instead of 0*inf=NaN
                nc.vector.tensor_scalar_add(out=rden[:], in0=ndl[:, TD:TD + H],
                                            scalar1=1e-30)
                nc.vector.reciprocal(out=rden[:], in_=rden[:])
                o_sb = opool.tile([P, TD], F16, tag="o_sb")
                nc.vector.tensor_tensor(
                    out=o_sb[:].rearrange("p (h d) -> p h d", h=H),
                    in0=ndl[:, 0:TD].rearrange("p (h d) -> p h d", h=H),
                    in1=rden[:, :, None].to_broadcast([P, H, D]),
                    op=ALU.mult)
                ps_oT = psC.tile([P, P], F16, tag="oT")
                nc.tensor.transpose(out=ps_oT[:], in_=o_sb[:], identity=c_id[:])
                oT_sb = opool.tile([P, P], F16, tag="oT_sb")
                nc.scalar.activation(out=oT_sb[:], in_=ps_oT[:], func=AF.Copy)
                ps_o = psC.tile([P, QD], F32, tag="ps_o")
                nc.tensor.matmul(out=ps_o[:], lhsT=oT_sb[:], rhs=c_WoT[:],
                                 start=True, stop=False)
                nc.tensor.matmul(out=ps_o[:], lhsT=c_ones[:], rhs=c_bo[:],
                                 start=False, stop=True)
                out_sb = opool.tile([P, QD], F32, tag="out_sb")
                nc.scalar.activation(out=out_sb[:], in_=ps_o[:], func=AF.Copy)
                nc.sync.dma_start(out=t_out[t * P:(t + 1) * P, :], in_=out_sb[:])

            emitted_a = [False]
            # ---- phase M: main loop over node tiles ----
            # gather batches group consecutive tiles
            batches = []
            t0 = 0
            while t0 < ntr:
                t1 = min(t0 + GB, ntr)
                batches.append((t0, t1))
                t0 = t1

            for (b0, b1) in (batches if STAGE != "proj" else []):
                e0, e1 = blk_off[b0] * P, blk_off[b1] * P
                ne = e1 - e0
                khv_g = gpool.tile([P, ne // P, 2 * TD], F16, tag="khv_g")
                nc.gpsimd.dma_gather(
                    out_ap=khv_g[:], in_ap=t_khv,
                    idxs_ap=c_dsti[:, e0 // 16:e1 // 16],
                    num_idxs=ne, num_idxs_reg=ne, elem_size=2 * TD,
                    single_packet=False)
                eT = gpool.tile([PD + 1, ne], F16, tag="eT")
                nc.sync.dma_start(out=eT[:], in_=t_eT[:, e0:e1])
                S_sb = gpool.tile([P, ne], F8, tag="S_sb")
                nc.sync.dma_start(out=S_sb[:], in_=t_S[:, e0:e1])
                ST_sb = gpool.tile([P, ne], F8, tag="ST_sb")
                nc.sync.dma_start(out=ST_sb[:], in_=t_ST[:, e0:e1])

                if b0 >= H1 and not emitted_a[0] and STAGE == "full":
                    emitted_a[0] = True
                    reduce_half(t_nd_a, t_ndr_a)
                    for tf in range(H1):
                        finalize(tf)
                for t in (range(b0, b1) if STAGE != "gather" else []):
                    nb = bpt[t]
                    ec = nb * P
                    go = blk_off[t] * P - e0     # edge offset in gather batch
                    gb = go // P                 # block offset in gather batch
                    # qh rows via one-hot matmul; qk product reads PSUM directly
                    prod = wpool.tile([P, nb, H, D], F16, tag="prod")
                    for b in range(nb):
                        ps_q = psA.tile([P, TD], F32, tag="proj")
                        nc.tensor.matmul(out=ps_q[:],
                                         lhsT=ST_sb[:, go + b * P:go + (b + 1) * P],
                                         rhs=qh_sb[:, t * TD:(t + 1) * TD],
                                         start=True, stop=True)
                        nc.vector.tensor_tensor(
                            out=prod[:, b, :, :],
                            in0=ps_q[:].rearrange("p (h d) -> p h d", h=H),
                            in1=khv_g[:, gb + b, 0:TD].rearrange("p (h d) -> p h d", h=H),
                            op=ALU.mult)
                    qk = wpool.tile([P, nb, H], F16, tag="qk")
                    with nc.allow_low_precision(reason="f16 qk logits are within tolerance"):
                        nc.vector.reduce_sum(out=qk[:], in_=prod[:], axis=mybir.AxisListType.X)
                    # edge-feature logits on PE: eb[e, h] per block
                    ps_eb = psB.tile([P, nb * H], F32, tag="eb")
                    for b in range(nb):
                        nc.tensor.matmul(out=ps_eb[:, b * H:(b + 1) * H],
                                         lhsT=eT[:, go + b * P:go + (b + 1) * P],
                                         rhs=c_Wb[:], start=True, stop=True)
                    attn = wpool.tile([P, nb * H], F32, tag="attn")
                    nc.vector.tensor_tensor(out=attn[:], in0=qk[:].rearrange("p b h -> p (b h)"),
                                            in1=ps_eb[:], op=ALU.add)
                    w_t = wpool.tile([P, nb, H], F16, tag="w")
                    nc.scalar.activation(out=w_t[:].rearrange("p b h -> p (b h)"),
                                         in_=attn[:], func=AF.Exp)
                    if STAGE == "attn":
                        continue
                    # rhs = [w*vh | w]
                    wv = wpool.tile([P, nb, 136], F16, tag="wv")
                    nc.vector.tensor_tensor(
                        out=wv[:, :, 0:TD].rearrange("p b (h d) -> p b h d", h=H),
                        in0=khv_g[:, gb:gb + nb, TD:2 * TD].rearrange("p b (h d) -> p b h d", h=H),
                        in1=w_t[:, :, :, None].to_broadcast([P, nb, H, D]),
                        op=ALU.mult)
                    nc.vector.tensor_copy(out=wv[:, :, TD:TD + H], in_=w_t[:])
                    # scatter-accumulate into num|den psum
                    ps_nd = psB.tile([P, 136], F32, tag="nd")
                    for b in range(nb):
                        nc.tensor.matmul(out=ps_nd[:],
                                         lhsT=S_sb[:, go + b * P:go + (b + 1) * P],
                                         rhs=wv[:, b, :],
                                         start=(b == 0), stop=(b == nb - 1))
                    nd_sb = opool.tile([P, 136], F32, tag="nd_sb")
                    nc.scalar.activation(out=nd_sb[:], in_=ps_nd[:], func=AF.Copy)
                    if t < H1:
                        nc.sync.dma_start(out=t_nd_a[t * P:(t + 1) * P, :], in_=nd_sb[:])
                    else:
                        nc.sync.dma_start(
                            out=t_nd_b[(t - H1) * P:(t - H1 + 1) * P, :], in_=nd_sb[:])

            # ---- pairwise AllReduce of num/den (two halves, first overlaps) ----
            if STAGE == "full":
                if not emitted_a[0]:
                    emitted_a[0] = True
                    reduce_half(t_nd_a, t_ndr_a)
                    for tf in range(H1):
                        finalize(tf)
                if ntr > H1:
                    reduce_half(t_nd_b, t_ndr_b)
                    for t in range(H1, ntr):
                        finalize(t)

    nc.compile()
    return nc


_CACHE = {}
LAST_RUN = {}


def kernel(**inputs) -> np.ndarray:
    q = np.asarray(inputs["q"], np.float32)
    k = np.asarray(inputs["k"], np.float32)
    v = np.asarray(inputs["v"], np.float32)
    edges = np.asarray(inputs["edges"], np.float32)
    edge_index = np.asarray(inputs["edge_index"])
    Wq, Wk, Wv = inputs["Wq"], inputs["Wk"], inputs["Wv"]
    Wb, bb, Wo, bo = inputs["Wb"], inputs["bb"], inputs["Wo"], inputs["bo"]

    cores, consts, meta = prepare(q, k, v, edges, edge_index, Wq, Wk, Wv, Wb, bb, Wo, bo)
    N = meta["N"]
    ntr = meta["ntiles_row"]

    key = (q.shape, edges.shape, meta["NBLK"])
    if key not in _CACHE:
        _CACHE[key] = build_program(meta)
    nc = _CACHE[key]

    in_maps = []
    for core in range(R * C):
        m = dict(cores[core])
        m.update({kk: np.ascontiguousarray(vv) for kk, vv in consts.items()})
        in_maps.append({kk: np.ascontiguousarray(vv) for kk, vv in m.items()})

    import os
    if os.environ.get("KERNEL_SIM"):
        from concourse.bass_interp import MultiCoreSim
        sim = MultiCoreSim(nc, num_cores=R * C)
        for ci, core_sim in sim.cores.items():
            for name, arr in in_maps[ci].items():
                core_sim.tensor(name)[:] = arr
        sim.simulate(check_with_hw=False)
        results = [{"o_out": np.array(sim.cores[ci].tensor("o_out"))}
                   for ci in range(R * C)]
    else:
        trace = bool(os.environ.get("KERNEL_TRACE"))
        res = bass_utils.run_bass_kernel_spmd(nc, in_maps, core_ids=list(range(R * C)),
                                              trace=trace)
        LAST_RUN["res"] = res
        results = res.results

    # assemble: core (i, j=0) output has all ntr tiles of row i (both halves
    # reduced identically); use j=0 cores.
    out = np.zeros((meta["NPAD"], QD), np.float32)
    node_tile, node_slot = meta["node_tile"], meta["node_slot"]
    for i in range(R):
        o = results[i * C]["o_out"]  # [ntr*P, QD]
        out[i * ntr * P:(i + 1) * ntr * P] = o
    # map back to node ids
    full = np.zeros((N, QD), np.float32)
    rowpos = node_tile * P + node_slot
    full[:, :] = out[rowpos[np.arange(N)]]
    # zero-degree nodes: reference yields bo
    zd = meta["deg"] == 0
    if zd.any():
        full[zd] = np.asarray(bo, np.float32)[None, :]
    return full



# revision 2
# speedup vs baseline: 1.3106x; 1.3106x over previous
"""GNN edge-softmax attention kernel for 8 Trainium2 NeuronCores — v2.

Strategy (8-way src-tile partition, zero collectives, zero dma_gather):
  - Host packs nodes into 128-slot tiles balanced by edge count (392 tiles),
    assigns 49 tiles to each core (LPT), and routes every edge to the core
    owning its src tile.  Per-core tile order is sorted by edge count so the
    shared block layout (bpt = max over cores) wastes <3% padding.
  - The host pre-gathers RAW k[dst] / v[dst] rows per edge (pure data
    movement; all arithmetic stays on device) as f16, k-tile-split for
    accumulating matmuls.  Each core streams these sequentially - no
    indirect DMA, no dma_gather descriptor-generation bottleneck.
  - Per 128-edge block on device: PE projects kh_e / vh_e (f16 matmuls),
    one-hot-gathers qh rows (ST matmul), computes edge-feature logits (eb);
    DVE forms qk = sum_d qh*kh, adds eb, ACT exponentiates; DVE weights vh;
    PE scatter-accumulates num|den into PSUM via one-hot S.
  - Finalize runs inline per tile (normalize + output projection); each core
    writes only its own 49 tiles.  Host re-permutes rows to node ids.
"""

import math
import sys

import numpy as np

sys.path.insert(0, "/opt/trn_rl_repo")

import concourse.bacc as bacc
import concourse.bass as bass
import concourse.mybir as mybir
import concourse.tile as tile
from concourse import bass_utils

F16 = mybir.dt.float16
F8 = mybir.dt.float8e4
F32 = mybir.dt.float32

H = 8            # heads
D = 16           # head dim
TD = H * D       # 128
QD = 256         # q/k/v feature dim
PD = 64          # edge pair feature dim
NC = 8           # cores
P = 128

AF = mybir.ActivationFunctionType
ALU = mybir.AluOpType
DR = mybir.MatmulPerfMode.DoubleRow


def _f8(x):
    import ml_dtypes
    return np.asarray(x, np.float32).astype(ml_dtypes.float8_e4m3)


def prepare(q, k, v, edges, edge_index, Wq, Wk, Wv, Wb, bb, Wo, bo):
    N = q.shape[0]
    T = NC * math.ceil(N / (NC * P))             # global tiles, mult of NC
    TPC = T // NC                                # tiles per core

    src = np.asarray(edge_index[:, 0], dtype=np.int64)
    dst = np.asarray(edge_index[:, 1], dtype=np.int64)
    deg = np.bincount(src, minlength=N)

    # --- greedy node->tile packing balanced by edge count ---
    order = np.argsort(-deg, kind="stable")
    tile_cnt = np.zeros(T, dtype=np.int64)
    tile_edges = np.zeros(T, dtype=np.int64)
    node_tile = np.zeros(N, dtype=np.int32)
    node_slot = np.zeros(N, dtype=np.int32)
    import heapq
    heap = [(0, t) for t in range(T)]
    heapq.heapify(heap)
    for n in order:
        while True:
            e_cnt, t = heapq.heappop(heap)
            if tile_cnt[t] < P:
                break
        node_tile[n] = t
        node_slot[n] = tile_cnt[t]
        tile_cnt[t] += 1
        tile_edges[t] += deg[n]
        if tile_cnt[t] < P:
            heapq.heappush(heap, (tile_edges[t], t))

    # --- tile -> core (LPT) then per-core order by count desc ---
    t_order = np.argsort(-tile_edges, kind="stable")
    core_load = [(0, c) for c in range(NC)]
    heapq.heapify(core_load)
    core_tiles = [[] for _ in range(NC)]
    for t in t_order:
        load, c = heapq.heappop(core_load)
        core_tiles[c].append(t)
        heapq.heappush(core_load, (load + int(tile_edges[t]), c))
    # per-core local order: by edge count desc (aligns heavy tiles at the
    # same tloc across cores so the shared bpt is tight)
    for c in range(NC):
        core_tiles[c].sort(key=lambda t: -int(tile_edges[t]))
    tile_of = np.zeros((NC, TPC), dtype=np.int64)    # (core, tloc) -> tile
    core_of_tile = np.zeros(T, dtype=np.int64)
    tloc_of_tile = np.zeros(T, dtype=np.int64)
    for c in range(NC):
        for i, t in enumerate(core_tiles[c]):
            tile_of[c, i] = t
            core_of_tile[t] = c
            tloc_of_tile[t] = i

    cnt = np.zeros((NC, TPC), dtype=np.int64)
    for c in range(NC):
        cnt[c] = tile_edges[tile_of[c]]
    bpt = np.maximum(1, np.ceil(cnt.max(axis=0) / P).astype(np.int64))  # [TPC]
    blk_off = np.concatenate([[0], np.cumsum(bpt)])
    NBLK = int(blk_off[-1])
    ECAP = NBLK * P

    # --- per-core edge arrays ---
    import ml_dtypes
    F8NP = ml_dtypes.float8_e4m3
    E3NP = ml_dtypes.float8_e3m4
    k8 = np.asarray(k, np.float32).astype(np.float16)   # [N, QD]
    v8 = np.asarray(v, np.float32).astype(np.float16)
    e8 = np.asarray(edges, np.float32).astype(E3NP)     # [E, PD]

    edge_core = core_of_tile[node_tile[src]]
    edge_tloc = tloc_of_tile[node_tile[src]]

    cores = []
    for c in range(NC):
        mask = edge_core == c
        es, ed, et = src[mask], dst[mask], edge_tloc[mask]
        ordr = np.argsort(et, kind="stable")
        es, ed, et = es[ordr], ed[ordr], et[ordr]
        eidx = np.nonzero(mask)[0][ordr]
        # position per edge: tile t's edges fill from blk_off[t]*P
        pos = np.zeros(len(es), dtype=np.int64)
        start = 0
        for t in range(TPC):
            ccc = int(cnt[c, t])
            pos[start:start + ccc] = blk_off[t] * P + np.arange(ccc)
            start += ccc

        # per-edge raw k/v rows, k-tile-split layout [128, NBLK, 2, 128]
        kT = np.zeros((P, NBLK, 2, P), dtype=np.float16)
        vT = np.zeros((P, NBLK, 2, P), dtype=np.float16)
        kr = np.zeros((ECAP, QD), dtype=np.float16)
        vr = np.zeros((ECAP, QD), dtype=np.float16)
        kr[pos] = k8[ed]
        vr[pos] = v8[ed]
        # [ECAP, 256] -> [NBLK, 128e, 2, 128kp] -> [kp, b, s, e]
        kT[:] = kr.reshape(NBLK, P, 2, P).transpose(3, 0, 2, 1)
        vT[:] = vr.reshape(NBLK, P, 2, P).transpose(3, 0, 2, 1)
        del kr, vr

        edgesT = np.zeros((PD + 1, ECAP), dtype=E3NP)
        edgesT[PD, :] = 1.0
        edgesT[:PD, pos] = e8[eidx].T

        slot = node_slot[es]
        S_en = np.zeros((ECAP, P), dtype=F8NP)
        S_en[pos, slot] = 1.0
        S3 = S_en.reshape(NBLK, P, P)
        S_mat = np.ascontiguousarray(S3.transpose(1, 0, 2)).reshape(P, ECAP)
        ST_mat = np.ascontiguousarray(S3.transpose(2, 0, 1)).reshape(P, ECAP)
        del S_en, S3

        # q rows for this core's tiles, k-tile-split layout [128, TPC, 2, 128]
        qT = np.zeros((P, TPC, 2, P), dtype=np.float16)
        qr = np.zeros((TPC * P, QD), dtype=np.float16)
        own = core_of_tile[node_tile] == c
        rn = np.nonzero(own)[0]
        qpos = tloc_of_tile[node_tile[rn]] * P + node_slot[rn]
        qr[qpos] = np.asarray(q[rn], np.float32).astype(np.float16)
        qT[:] = qr.reshape(TPC, P, 2, P).transpose(3, 0, 2, 1)
        del qr

        cores.append(dict(kT=np.ascontiguousarray(kT.reshape(P, NBLK * 2 * P)),
                          vT=np.ascontiguousarray(vT.reshape(P, NBLK * 2 * P)),
                          edgesT=edgesT, S_mat=S_mat, ST_mat=ST_mat,
                          qT=np.ascontiguousarray(qT.reshape(P, TPC * 2 * P))))

    norm = D ** -0.5
    # weights in k-tile-split rhs layout [128kp, 2s, 128td]
    def w2(W, scale=1.0):
        Wf = np.asarray(W, np.float32) * scale      # [TD, QD]
        return np.ascontiguousarray(
            Wf.T.reshape(2, P, TD).transpose(1, 0, 2).astype(np.float16).reshape(P, 2 * TD))

    consts = dict(
        Wq2=w2(Wq, norm), Wk2=w2(Wk), Wv2=w2(Wv),
        WbT_aug=np.concatenate(
            [np.asarray(Wb, np.float32).T,
             np.asarray(bb, np.float32)[None, :]], axis=0).astype(np.float16),
        WoT=np.asarray(Wo, np.float32).T.astype(np.float16),
        bo_row=np.asarray(bo, np.float32)[None, :].astype(np.float16),
        identity=np.eye(P, dtype=np.float16),
        ones_col=np.ones((1, P), dtype=np.float16),
    )
    meta = dict(N=N, T=T, TPC=TPC, NBLK=NBLK, ECAP=ECAP,
                bpt=bpt.tolist(), blk_off=blk_off.tolist(),
                node_tile=node_tile, node_slot=node_slot, deg=deg,
                core_of_tile=core_of_tile, tloc_of_tile=tloc_of_tile)
    return cores, consts, meta


def build_program(meta, gather_batch=2):
    TPC = meta["TPC"]
    NBLK, ECAP = meta["NBLK"], meta["ECAP"]
    bpt, blk_off = meta["bpt"], meta["blk_off"]

    nc = bacc.Bacc("TRN2", target_bir_lowering=False, debug=False, num_devices=NC)
    dt = nc.dram_tensor
    t_kT = dt("kT", [P, NBLK * 2 * P], F16, kind="ExternalInput").ap()
    t_vT = dt("vT", [P, NBLK * 2 * P], F16, kind="ExternalInput").ap()
    t_eT = dt("edgesT", [PD + 1, ECAP], mybir.dt.float8e3, kind="ExternalInput").ap()
    t_S = dt("S_mat", [P, ECAP], F8, kind="ExternalInput").ap()
    t_ST = dt("ST_mat", [P, ECAP], F8, kind="ExternalInput").ap()
    t_qT = dt("qT", [P, TPC * 2 * P], F16, kind="ExternalInput").ap()
    t_Wq2 = dt("Wq2", [P, 2 * TD], F16, kind="ExternalInput").ap()
    t_Wk2 = dt("Wk2", [P, 2 * TD], F16, kind="ExternalInput").ap()
    t_Wv2 = dt("Wv2", [P, 2 * TD], F16, kind="ExternalInput").ap()
    t_Wb = dt("WbT_aug", [PD + 1, H], F16, kind="ExternalInput").ap()
    t_WoT = dt("WoT", [TD, QD], F16, kind="ExternalInput").ap()
    t_bo = dt("bo_row", [1, QD], F16, kind="ExternalInput").ap()
    t_id = dt("identity", [P, P], F16, kind="ExternalInput").ap()
    t_ones = dt("ones_col", [1, P], F16, kind="ExternalInput").ap()
    t_out = dt("o_out", [TPC * P, QD], F16, kind="ExternalOutput").ap()

    GB = gather_batch

    with tile.TileContext(nc) as tc:
        with (
            tc.tile_pool(name="const", bufs=1) as cpool,
            tc.tile_pool(name="gath", bufs=3) as gpool,
            tc.tile_pool(name="work", bufs=3) as wpool,
            tc.tile_pool(name="out", bufs=3) as opool,
            tc.tile_pool(name="psQ", bufs=2, space="PSUM") as psQ,
            tc.tile_pool(name="psKV", bufs=2, space="PSUM") as psKV,
            tc.tile_pool(name="psNE", bufs=2, space="PSUM") as psNE,
        ):
            # ---- constants ----
            c_Wq2 = cpool.tile([P, 2 * TD], F16); nc.sync.dma_start(out=c_Wq2[:], in_=t_Wq2)
            c_Wk2 = cpool.tile([P, 2 * TD], F16); nc.sync.dma_start(out=c_Wk2[:], in_=t_Wk2)
            c_Wv2 = cpool.tile([P, 2 * TD], F16); nc.sync.dma_start(out=c_Wv2[:], in_=t_Wv2)
            c_Wb = cpool.tile([PD + 1, H], F16); nc.sync.dma_start(out=c_Wb[:], in_=t_Wb)
            c_WoT = cpool.tile([TD, QD], F16); nc.sync.dma_start(out=c_WoT[:], in_=t_WoT)
            c_bo = cpool.tile([1, QD], F16); nc.sync.dma_start(out=c_bo[:], in_=t_bo)
            c_id = cpool.tile([P, P], F16); nc.sync.dma_start(out=c_id[:], in_=t_id)
            c_ones = cpool.tile([1, P], F16); nc.sync.dma_start(out=c_ones[:], in_=t_ones)
            qh_sb = cpool.tile([P, TPC * TD], F16)

            # ---- qh projection (f16, 2 accumulating matmuls) ----
            for g0 in range(0, TPC, 8):
                g1 = min(g0 + 8, TPC)
                qin = gpool.tile([P, 8 * 2 * P], F16, tag="qin")
                nc.sync.dma_start(out=qin[:, :(g1 - g0) * 2 * P],
                                  in_=t_qT[:, g0 * 2 * P:g1 * 2 * P])
                qv = qin[:, :(g1 - g0) * 2 * P].rearrange("p (t s e) -> p t s e", s=2, e=P)
                for t in range(g0, g1):
                    ps = psQ.tile([P, TD], F32, tag="q")
                    nc.tensor.matmul(out=ps[:], lhsT=qv[:, t - g0, 0, :],
                                     rhs=c_Wq2[:, 0:TD], start=True, stop=False)
                    nc.tensor.matmul(out=ps[:], lhsT=qv[:, t - g0, 1, :],
                                     rhs=c_Wq2[:, TD:2 * TD], start=False, stop=True)
                    nc.scalar.activation(out=qh_sb[:, t * TD:(t + 1) * TD],
                                         in_=ps[:], func=AF.Copy)

            # ---- main loop over tiles ----
            batches = []
            t0 = 0
            while t0 < TPC:
                t1 = min(t0 + GB, TPC)
                batches.append((t0, t1))
                t0 = t1
            BW = max(blk_off[t1] - blk_off[t0] for (t0, t1) in batches) * P
            MAXNB = max(bpt)

            for (b0, b1) in batches:
                e0, e1 = blk_off[b0] * P, blk_off[b1] * P
                ne = e1 - e0
                kT_sb = gpool.tile([P, 2 * BW], F16, tag="kT")
                nc.sync.dma_start(out=kT_sb[:, :2 * ne], in_=t_kT[:, 2 * e0:2 * e1])
                vT_sb = gpool.tile([P, 2 * BW], F16, tag="vT")
                nc.sync.dma_start(out=vT_sb[:, :2 * ne], in_=t_vT[:, 2 * e0:2 * e1])
                eT_sb = gpool.tile([PD + 1, BW], mybir.dt.float8e3, tag="eT")
                nc.sync.dma_start(out=eT_sb[:, :ne], in_=t_eT[:, e0:e1])
                S_sb = gpool.tile([P, BW], F8, tag="S")
                nc.sync.dma_start(out=S_sb[:, :ne], in_=t_S[:, e0:e1])
                ST_sb = gpool.tile([P, BW], F8, tag="ST")
                nc.sync.dma_start(out=ST_sb[:, :ne], in_=t_ST[:, e0:e1])
                kv_v = kT_sb[:, :2 * ne].rearrange("p (b s e) -> p b s e", s=2, e=P)
                vv_v = vT_sb[:, :2 * ne].rearrange("p (b s e) -> p b s e", s=2, e=P)

                for t in range(b0, b1):
                    nb = bpt[t]
                    go = blk_off[t] * P - e0
                    gb = go // P
                    ngrp = (nb + 3) // 4
                    # ne-bank layout (f32 cols): nd [0:136], eb [136:136+8nb],
                    # transpose scratch f16 [280:344], ps_o reuses [0:256]
                    ps_ne = psNE.tile([P, 344], F32, tag="ne")
                    wv = wpool.tile([P, MAXNB, 136], F16, tag="wv")
                    for g in range(ngrp):
                        gs = min(4, nb - g * 4)
                        ps_q4 = psQ.tile([P, 4, TD], F32, tag="q")
                        ps_kv4 = psKV.tile([P, 4, 2, TD], F32, tag="kv")
                        for j in range(gs):
                            b = g * 4 + j
                            nc.tensor.matmul(out=ps_kv4[:, j, 0, :],
                                             lhsT=kv_v[:, gb + b, 0, :],
                                             rhs=c_Wk2[:, 0:TD], start=True, stop=False)
                            nc.tensor.matmul(out=ps_kv4[:, j, 0, :],
                                             lhsT=kv_v[:, gb + b, 1, :],
                                             rhs=c_Wk2[:, TD:2 * TD], start=False, stop=True)
                            nc.tensor.matmul(out=ps_kv4[:, j, 1, :],
                                             lhsT=vv_v[:, gb + b, 0, :],
                                             rhs=c_Wv2[:, 0:TD], start=True, stop=False)
                            nc.tensor.matmul(out=ps_kv4[:, j, 1, :],
                                             lhsT=vv_v[:, gb + b, 1, :],
                                             rhs=c_Wv2[:, TD:2 * TD], start=False, stop=True)
                            nc.tensor.matmul(out=ps_q4[:, j, :],
                                             lhsT=ST_sb[:, go + b * P:go + (b + 1) * P],
                                             rhs=qh_sb[:, t * TD:(t + 1) * TD],
                                             start=True, stop=True)
                            nc.tensor.matmul(out=ps_ne[:, 136 + b * H:136 + (b + 1) * H],
                                             lhsT=eT_sb[:, go + b * P:go + (b + 1) * P],
                                             rhs=c_Wb[:], start=True, stop=True)
                        # ACT evacuates kh|vh for the group in one copy
                        khv4 = wpool.tile([P, 4, 2, TD], F16, tag="khv4")
                        nc.scalar.activation(out=khv4[:, 0:gs, :, :],
                                             in_=ps_kv4[:, 0:gs, :, :], func=AF.Copy)
                        prod = wpool.tile([P, 4, H, D], F16, tag="prod")
                        nc.vector.tensor_tensor(
                            out=prod[:, 0:gs, :, :],
                            in0=ps_q4[:, 0:gs, :].rearrange("p b (h d) -> p b h d", h=H),
                            in1=khv4[:, 0:gs, 0, :].rearrange("p b (h d) -> p b h d", h=H),
                            op=ALU.mult)
                        qk4 = wpool.tile([P, 4, H], F16, tag="qk4")
                        with nc.allow_low_precision(reason="f16 qk logits"):
                            nc.vector.reduce_sum(
                                out=qk4[:, 0:gs, :].rearrange("p b h -> p (b h)"),
                                in_=prod[:, 0:gs, :, :].rearrange("p b h d -> p (b h) d"),
                                axis=mybir.AxisListType.X)
                        attn4 = wpool.tile([P, 4 * H], F32, tag="attn4")
                        nc.vector.tensor_tensor(
                            out=attn4[:, 0:gs * H],
                            in0=qk4[:, 0:gs, :].rearrange("p b h -> p (b h)"),
                            in1=ps_ne[:, 136 + g * 4 * H:136 + (g * 4 + gs) * H],
                            op=ALU.add)
                        # exp with broadcast-expanded output: w4x[e, b, h, d] = w[e, b, h]
                        w4x = wpool.tile([P, 4, H, D], F16, tag="w4x")
                        nc.scalar.activation(
                            out=w4x[:, 0:gs, :, :],
                            in_=attn4[:, 0:gs * H].rearrange(
                                "p (b h) -> p b h", h=H)[:, :, :, None].to_broadcast(
                                [P, gs, H, D]),
                            func=AF.Exp)
                        # weighted vh: both operands packed f16 (2x mode)
                        nc.vector.tensor_tensor(
                            out=wv[:, g * 4:g * 4 + gs, 0:TD],
                            in0=khv4[:, 0:gs, 1, :],
                            in1=w4x[:, 0:gs, :, :].rearrange("p b h d -> p b (h d)"),
                            op=ALU.mult)
                        nc.vector.tensor_copy(out=wv[:, g * 4:g * 4 + gs, TD:TD + H],
                                              in_=w4x[:, 0:gs, :, 0])
                    # scatter after ALL eb groups closed (same bank: the nd
                    # accumulation group must not interleave with eb starts)
                    for b in range(nb):
                        nc.tensor.matmul(out=ps_ne[:, 0:136],
                                         lhsT=S_sb[:, go + b * P:go + (b + 1) * P],
                                         rhs=wv[:, b, :],
                                         start=(b == 0), stop=(b == nb - 1))
                    # ---- finalize tile t inline (reusing the ne bank) ----
                    ndl = opool.tile([P, 136], F32, tag="ndl")
                    nc.vector.tensor_copy(out=ndl[:], in_=ps_ne[:, 0:136])
                    rden = opool.tile([P, H], F32, tag="rden")
                    nc.vector.tensor_scalar_add(out=rden[:], in0=ndl[:, TD:TD + H],
                                                scalar1=1e-30)
                    nc.vector.reciprocal(out=rden[:], in_=rden[:])
                    o_sb = opool.tile([P, TD], F16, tag="o_sb")
                    nc.vector.tensor_tensor(
                        out=o_sb[:].rearrange("p (h d) -> p h d", h=H),
                        in0=ndl[:, 0:TD].rearrange("p (h d) -> p h d", h=H),
                        in1=rden[:, :, None].to_broadcast([P, H, D]),
                        op=ALU.mult)
                    ps_oT = ps_ne[:, 280:344].bitcast(F16)      # [P, 128] f16
                    nc.tensor.transpose(out=ps_oT, in_=o_sb[:], identity=c_id[:])
                    oT_sb = opool.tile([P, P], F16, tag="oT_sb")
                    nc.vector.tensor_copy(out=oT_sb[:], in_=ps_oT)
                    ps_o = ps_ne[:, 0:256]
                    nc.tensor.matmul(out=ps_o, lhsT=oT_sb[:], rhs=c_WoT[:],
                                     start=True, stop=False)
                    nc.tensor.matmul(out=ps_o, lhsT=c_ones[:], rhs=c_bo[:],
                                     start=False, stop=True)
                    out_sb = opool.tile([P, QD], F16, tag="out_sb")
                    nc.vector.tensor_copy(out=out_sb[:], in_=ps_o)
                    nc.sync.dma_start(out=t_out[t * P:(t + 1) * P, :], in_=out_sb[:])

    nc.compile()
    return nc


_CACHE = {}
LAST_RUN = {}


def kernel(**inputs) -> np.ndarray:
    q = np.asarray(inputs["q"], np.float32)
    k = np.asarray(inputs["k"], np.float32)
    v = np.asarray(inputs["v"], np.float32)
    edges = np.asarray(inputs["edges"], np.float32)
    edge_index = np.asarray(inputs["edge_index"])
    Wq, Wk, Wv = inputs["Wq"], inputs["Wk"], inputs["Wv"]
    Wb, bb, Wo, bo = inputs["Wb"], inputs["bb"], inputs["Wo"], inputs["bo"]

    cores, consts, meta = prepare(q, k, v, edges, edge_index,
                                  Wq, Wk, Wv, Wb, bb, Wo, bo)
    N = meta["N"]
    TPC = meta["TPC"]

    key = (q.shape, edges.shape, meta["NBLK"])
    if key not in _CACHE:
        _CACHE[key] = build_program(meta)
    nc = _CACHE[key]

    in_maps = []
    for c in range(NC):
        m = dict(cores[c])
        m.update({kk: np.ascontiguousarray(vv) for kk, vv in consts.items()})
        in_maps.append(m)

    import os
    if os.environ.get("KERNEL_SIM"):
        from concourse.bass_interp import MultiCoreSim
        sim = MultiCoreSim(nc, num_cores=NC)
        for ci, core_sim in sim.cores.items():
            for name, arr in in_maps[ci].items():
                core_sim.tensor(name)[:] = arr
        sim.simulate(check_with_hw=False)
        results = [{"o_out": np.array(sim.cores[ci].tensor("o_out"))}
                   for ci in range(NC)]
    else:
        res = bass_utils.run_bass_kernel_spmd(nc, in_maps, core_ids=list(range(NC)))
        LAST_RUN["res"] = res
        results = res.results

    # assemble
    full = np.zeros((N, QD), np.float32)
    node_tile, node_slot = meta["node_tile"], meta["node_slot"]
    core_of_tile, tloc_of_tile = meta["core_of_tile"], meta["tloc_of_tile"]
    outs = [np.asarray(results[c]["o_out"], np.float32) for c in range(NC)]
    nt = node_tile[np.arange(N)]
    rows = tloc_of_tile[nt] * P + node_slot[np.arange(N)]
    for c in range(NC):
        m = core_of_tile[nt] == c
        full[m] = outs[c][rows[m]]
    zd = meta["deg"] == 0
    if zd.any():
        full[zd] = np.asarray(bo, np.float32)[None, :]
    return full
